# revision 1
# baseline (speedup 1.0000x reference)
"""Trainium2 Bass kernel for nn_BottomUpIntegrator (gnn_message_passing).

Sharding: cells split at cluster boundaries across 8 cores (2048 clusters each);
per-core segment sums via one-hot scatter matmuls into PSUM with a core-invariant
static window schedule; cluster+organism phase on-chip; host combines 12 organism
partial floats per core into the final 6 self-model outputs.
"""
import numpy as np
import ml_dtypes

import json as _json

from concourse import bass, mybir
from concourse import bass2jax as _b2j
from concourse import bass_utils as _bu
from concourse.tile import TileContext
from concourse.bass_utils import run_bass_kernel_spmd

_orig_compile = _bu.compile_bir_kernel


def _split_waits_compile(bir_json, tmpdir, neff_name="file.neff"):
    """Walrus lowers at most ONE semaphore wait per TPB instruction struct.
    Tile emits several. Hoist extras onto injected same-engine EventSemaphore
    wait instructions immediately before the owner (semantically identical:
    engines execute in program order)."""
    d = _json.loads(bir_json)
    cnt = 0
    for fn in d["functions"]:
        for blk in fn["blocks"]:
            newlist = []
            for ins in blk["instructions"]:
                si = ins.get("sync_info")
                waits = si.get("on_wait", []) if si else []
                if si and len(waits) > 1 and ins.get("opcode") not in (
                        "EventSemaphore",):
                    for w_i, w in enumerate(waits[:-1]):
                        cnt += 1
                        newlist.append({
                            "debug": ins.get("debug", 0),
                            "engine": ins["engine"],
                            "ins": [], "outs": [],
                            "name": f"{ins['name']}-wsplit{w_i}",
                            "opcode": "EventSemaphore",
                            "sync_info": {"on_update": [], "on_wait": [w]},
                        })
                    si["on_wait"] = [waits[-1]]
                newlist.append(ins)
            blk["instructions"] = newlist
    print(f"[wait-split] hoisted {cnt} extra waits")
    return _orig_compile(_json.dumps(d).encode(), tmpdir, neff_name=neff_name)


_bu.compile_bir_kernel = _split_waits_compile
_b2j.compile_bir_kernel = _split_waits_compile

F32 = mybir.dt.float32
BF16 = mybir.dt.bfloat16
AF = mybir.ActivationFunctionType
OP = mybir.AluOpType
AX = mybir.AxisListType

NCORES = 8
KLOC = 2048            # clusters per core
NPAD = 262144          # padded cells per core
CHUNK = 8192           # cells per chunk
NCHUNK = NPAD // CHUNK # 16
W = 32                 # onehot window width (clusters)
NTILES = NPAD // 128   # 2048 scatter tiles per core
TPB = NTILES // 4      # tiles per 512-cluster block
PADSEG = 1.0e9


def _window_start(S):
    s = S % TPB
    return int(np.clip(s - 16, 0, 512 - W))


def build_program():
    nc = bass.Bass(trn_type="TRN2", use_seq_codegen=True)
    featsT = nc.dram_tensor("featsT", [72, NPAD // 2], BF16, kind="ExternalInput")
    archcm = nc.dram_tensor("archcm", [NCHUNK, 128, 256], F32, kind="ExternalInput")
    cellvec = nc.dram_tensor("cellvec", [NCHUNK, 128, 256], F32, kind="ExternalInput")
    w1d = nc.dram_tensor("w1d", [72, 128], BF16, kind="ExternalInput")
    b1d = nc.dram_tensor("b1d", [128, 1], F32, kind="ExternalInput")
    w2d = nc.dram_tensor("w2d", [128, 2], BF16, kind="ExternalInput")
    b2d = nc.dram_tensor("b2d", [128, 1], F32, kind="ExternalInput")
    iotat = nc.dram_tensor("iotat", [128, 32 * 128], F32, kind="ExternalInput")
    ident = nc.dram_tensor("ident", [128, 128], F32, kind="ExternalInput")
    v1 = nc.dram_tensor("v1", [7, 32], F32, kind="ExternalInput")
    c1b = nc.dram_tensor("c1b", [32, 1], F32, kind="ExternalInput")
    v2 = nc.dram_tensor("v2", [32, 1], F32, kind="ExternalInput")
    c2b = nc.dram_tensor("c2b", [1, 1], F32, kind="ExternalInput")
    out_cluster = nc.dram_tensor("out_cluster", [KLOC, 8], F32, kind="ExternalOutput")
    out_org = nc.dram_tensor("out_org", [1, 12], F32, kind="ExternalOutput")

    with TileContext(nc) as tc:
        with (
            tc.tile_pool(name="const", bufs=1) as cp,
            tc.tile_pool(name="feats", bufs=2) as fp,
            tc.tile_pool(name="hs", bufs=2) as hp,
            tc.tile_pool(name="small", bufs=4) as sp,
            tc.tile_pool(name="scatv", bufs=2) as vp,
            tc.tile_pool(name="ph_b", bufs=1) as bp,
            tc.tile_pool(name="scatps", bufs=1, space="PSUM") as pps,
        ):
            # ---- constants ----------------------------------------------
            w1s = cp.tile([72, 128], BF16, tag="w1s")
            nc.sync.dma_start(out=w1s[:], in_=w1d[:])
            b1s = cp.tile([128, 1], F32, tag="b1s")
            nc.sync.dma_start(out=b1s[:], in_=b1d[:])
            w2s = cp.tile([128, 2], BF16, tag="w2s")
            nc.sync.dma_start(out=w2s[:], in_=w2d[:])
            b2s = cp.tile([128, 1], F32, tag="b2s")
            nc.sync.dma_start(out=b2s[:], in_=b2d[:])
            iots = cp.tile([128, 32 * 128], F32, tag="iots")
            nc.sync.dma_start(out=iots[:], in_=iotat[:])
            ids = cp.tile([128, 128], F32, tag="ids")
            nc.sync.dma_start(out=ids[:], in_=ident[:])
            v1s = cp.tile([7, 32], F32, tag="v1s")
            nc.sync.dma_start(out=v1s[:], in_=v1[:])
            c1s = cp.tile([32, 1], F32, tag="c1s")
            nc.sync.dma_start(out=c1s[:], in_=c1b[:])
            v2s = cp.tile([32, 1], F32, tag="v2s")
            nc.sync.dma_start(out=v2s[:], in_=v2[:])
            c2s = cp.tile([1, 1], F32, tag="c2s")
            nc.sync.dma_start(out=c2s[:], in_=c2b[:])
            ones = cp.tile([128, 1], F32, tag="ones")
            nc.vector.memset(ones[:], 1.0)

            zbf = cp.tile([128, 512], BF16, tag="zbf")
            nc.vector.memset(zbf[:], 0.0)

            # Pre-touch DMA-loaded constants on their consuming engines so no
            # later compute instruction needs a second (DMA) semaphore wait —
            # the MM/ACT ISA structs hold only one wait.
            scra = cp.tile([128, 4], F32, tag="scra")
            nc.scalar.activation(out=scra[:, 0:1], in_=b1s[:], func=AF.Copy)
            nc.scalar.activation(out=scra[:, 1:2], in_=b2s[:], func=AF.Copy)
            nc.scalar.activation(out=scra[0:32, 2:3], in_=c1s[:], func=AF.Copy)
            nc.scalar.activation(out=scra[0:1, 3:4], in_=c2s[:], func=AF.Copy)
            scrv = cp.tile([1, 1], F32, tag="scrv")
            nc.vector.tensor_copy(out=scrv[:], in_=iots[0:1, 0:1])

            # persistent scatter accumulators: 4 PSUM banks of [15, 512],
            # zero-initialized by an all-zero matmul (sets the bank's
            # accumulation group; subsequent scatter matmuls accumulate).
            scat = [pps.tile([15, 512], F32, tag=f"scat{b}", name=f"scat{b}")
                    for b in range(4)]
            # PE touch of the identity const (rides on Ldweights; overwritten
            # by the zeroing matmul below).
            nc.tensor.matmul(out=scat[0][0:1, 0:1], lhsT=ids[0:1, 0:1],
                             rhs=ids[0:1, 0:1], start=True, stop=True,
                             skip_group_check=True)
            for b in range(4):
                nc.tensor.matmul(out=scat[b][:], lhsT=zbf[:, 0:15], rhs=zbf[:],
                                 start=True, stop=False, skip_group_check=True)

            # ---- phase A ------------------------------------------------
            with (
                tc.tile_pool(name="mm1ps", bufs=2, space="PSUM") as pp1,
                tc.tile_pool(name="mm2ps", bufs=2, space="PSUM") as pp2,
            ):
                for k in range(NCHUNK):
                    if k > 0 and k % 2 == 0:
                        # SP-side sync point: absorbs cross-chunk WAR/WAW
                        # waits so every compute/DMA instruction needs at
                        # most ONE semaphore wait (the ISA struct limit).
                        tc.strict_bb_all_engine_barrier()
                    c0 = k * CHUNK
                    ft = fp.tile([72, 4096], BF16, tag="ft")
                    nc.sync.dma_start(out=ft[:],
                                      in_=featsT[:, k * 4096:(k + 1) * 4096])
                    ac = sp.tile([128, 256], F32, tag="ac")
                    nc.sync.dma_start(out=ac[:], in_=archcm[k])
                    cv = sp.tile([128, 256], F32, tag="cv")
                    nc.sync.dma_start(out=cv[:], in_=cellvec[k])
                    # pad to 4 DMAs/chunk so the 8-lane round-robin reassigns
                    # the SAME lane to each tensor slot (reused at k-2): keeps
                    # every DMA at <=2 sync waits (the HW struct limit).
                    dmy = sp.tile([1, 1], F32, tag="dmy")
                    nc.sync.dma_start(out=dmy[:], in_=c2b[:])

                    # mm1 + bias + relu -> h [128, 8192] bf16
                    hs = hp.tile([128, 4096], BF16, tag="hs")
                    for j in range(8):
                        hp1 = pp1.tile([128, 512], F32, tag="hp1")
                        nc.tensor.matmul(out=hp1[:], lhsT=w1s[:],
                                         rhs=ft[:, 512 * j:512 * (j + 1)],
                                         start=True, stop=True)
                        nc.scalar.activation(out=hs[:, 512 * j:512 * (j + 1)],
                                             in_=hp1[:], func=AF.Relu, bias=b1s[:])

                    # mm2 -> base [128, 128] cell-major in PSUM
                    bb = pp2.tile([128, 64], F32, tag="bb")
                    for t in range(32):
                        nc.tensor.matmul(out=bb[:, 2 * t:2 * t + 2],
                                         lhsT=hs[:, 128 * t:128 * (t + 1)], rhs=w2s[:],
                                         start=(t == 0), stop=(t == 31),
                                         skip_group_check=True)

                    # chain: ew = exp(clip(sig(base)*e*phi,.01,1)*e*phi)
                    sg = sp.tile([128, 64], F32, tag="sg")
                    nc.scalar.activation(out=sg[:], in_=bb[:], func=AF.Sigmoid,
                                         bias=b2s[:])
                    eph = sp.tile([128, 64], F32, tag="eph")
                    nc.vector.tensor_tensor(out=eph[:], in0=cv[:, 0:64],
                                            in1=cv[:, 64:128], op=OP.mult)
                    imp = sp.tile([128, 64], F32, tag="imp")
                    nc.vector.tensor_tensor(out=imp[:], in0=sg[:], in1=eph[:],
                                            op=OP.mult)
                    nc.vector.tensor_scalar(out=imp[:], in0=imp[:], scalar1=0.01,
                                            scalar2=1.0, op0=OP.max, op1=OP.min)
                    wc = sp.tile([128, 64], F32, tag="wc")
                    nc.vector.tensor_tensor(out=wc[:], in0=imp[:], in1=eph[:],
                                            op=OP.mult)
                    ew = sp.tile([128, 64], F32, tag="ew")
                    nc.scalar.activation(out=ew[:], in_=wc[:], func=AF.Exp)

                    # values [128, 15*128] bf16 + onehot [128, 32*128] bf16
                    vt = vp.tile([128, 15 * 64], BF16, tag="vt")
                    vv = vt[:].rearrange("p (s v) -> p s v", v=15)
                    acv = ac[:].rearrange("p (s a) -> p s a", a=4)
                    nc.vector.memset(vv[:, :, 0:1], 1.0)
                    nc.vector.tensor_copy(out=vv[:, :, 1:2],
                                          in_=ew[:].to_broadcast([128, 64, 1]))
                    nc.vector.tensor_tensor(out=vv[:, :, 2:6], in0=acv,
                                            in1=ew[:].to_broadcast([128, 64, 4]),
                                            op=OP.mult)
                    nc.vector.tensor_copy(out=vv[:, :, 6:10], in_=acv)
                    nc.vector.tensor_tensor(out=vv[:, :, 10:14], in0=acv, in1=acv,
                                            op=OP.mult)
                    nc.vector.tensor_copy(out=vv[:, :, 14:15],
                                          in_=cv[:, 128:192].to_broadcast([128, 64, 1]))
                    oh = vp.tile([128, 32 * 64], BF16, tag="oh")
                    ohv = oh[:].rearrange("p (s w) -> p s w", w=32)
                    iov = iots[:, 0:32 * 64].rearrange("p (s w) -> p s w", w=32)
                    nc.vector.tensor_tensor(out=ohv, in0=iov,
                                            in1=cv[:, 192:256].to_broadcast([128, 64, 32]),
                                            op=OP.is_equal)

                    # scatter: col j -> sorted tile S = 128k + 64*(j%2) + j//2
                    for j in range(64):
                        S = 64 * k + 32 * (j % 2) + (j // 2)
                        blk = S // TPB
                        f = _window_start(S)
                        nc.tensor.matmul(out=scat[blk][:, f:f + W],
                                         lhsT=vt[:, 15 * j:15 * j + 15],
                                         rhs=oh[:, 32 * j:32 * j + 32],
                                         start=False,
                                         stop=(k == NCHUNK - 1 and j >= 62),
                                         skip_group_check=True)

            # ---- phase B ------------------------------------------------
            tc.strict_bb_all_engine_barrier()
            sc = bp.tile([15, 2048], F32, tag="sc")
            for b in range(4):
                nc.vector.tensor_copy(out=sc[:, 512 * b:512 * (b + 1)], in_=scat[b][:])

            with (
                tc.tile_pool(name="ptps", bufs=2, space="PSUM") as ppt,
                tc.tile_pool(name="mmbps", bufs=2, space="PSUM") as ppm,
            ):
                tt = bp.tile([128, 16 * 15], F32, tag="tt")
                for b in range(16):
                    pt = ppt.tile([128, 15], F32, tag="pt")
                    nc.tensor.transpose(out=pt[:], in_=sc[:, 128 * b:128 * (b + 1)],
                                        identity=ids[0:15, 0:15])
                    nc.vector.tensor_copy(out=tt[:, 15 * b:15 * (b + 1)], in_=pt[:])
                tv = tt[:].rearrange("p (b q) -> p b q", q=15)
                cnt = tv[:, :, 0:1]      # [128,16,1]
                sew = tv[:, :, 1:2]
                sewa = tv[:, :, 2:6]
                sa = tv[:, :, 6:10]
                ssq = tv[:, :, 10:14]
                ssur = tv[:, :, 14:15]

                def wt(tag):
                    return bp.tile([128, 16], F32, tag=tag, name=tag)

                def v3(t):
                    return t[:].rearrange("p (b a) -> p b a", a=1)

                def w4(tag):
                    t = bp.tile([128, 64], F32, tag=tag, name=tag)
                    return t, t[:].rearrange("p (b a) -> p b a", a=4)

                cntc = wt("cntc")
                nc.vector.tensor_scalar(out=v3(cntc), in0=cnt, scalar1=1.0,
                                        scalar2=None, op0=OP.max)
                rc = wt("rc")
                nc.vector.reciprocal(out=rc[:], in_=cntc[:])
                den = wt("den")
                nc.vector.tensor_scalar(out=v3(den), in0=sew, scalar1=1.0,
                                        scalar2=None, op0=OP.max)
                rden = wt("rden")
                nc.vector.reciprocal(out=rden[:], in_=den[:])
                agr, agrv = w4("agr")
                nc.vector.tensor_tensor(out=agrv, in0=sewa,
                                        in1=rden[:].to_broadcast([128, 16, 4]),
                                        op=OP.mult)
                mx = wt("mx")
                nc.vector.tensor_reduce(out=v3(mx), in_=agrv, axis=AX.X, op=OP.max)
                es, esv = w4("es")
                nc.vector.tensor_tensor(out=esv, in0=agrv,
                                        in1=mx[:].to_broadcast([128, 16, 4]),
                                        op=OP.subtract)
                nc.scalar.activation(out=es[:], in_=es[:], func=AF.Exp)
                ssum = wt("ssum")
                nc.vector.tensor_reduce(out=v3(ssum), in_=esv, axis=AX.X, op=OP.add)
                rssum = wt("rssum")
                nc.vector.reciprocal(out=rssum[:], in_=ssum[:])
                agg, aggv = w4("agg")
                nc.vector.tensor_tensor(out=aggv, in0=esv,
                                        in1=rssum[:].to_broadcast([128, 16, 4]),
                                        op=OP.mult)
                mean, meanv = w4("mean")
                nc.vector.tensor_tensor(out=meanv, in0=sa,
                                        in1=rc[:].to_broadcast([128, 16, 4]),
                                        op=OP.mult)
                var, varv = w4("var")
                nc.vector.tensor_tensor(out=varv, in0=meanv, in1=meanv, op=OP.mult)
                cntb = wt("cntb")
                nc.vector.tensor_copy(out=v3(cntb), in_=cnt)
                nc.vector.tensor_tensor(out=varv, in0=varv,
                                        in1=cntb[:].to_broadcast([128, 16, 4]),
                                        op=OP.mult)
                nc.vector.tensor_tensor(out=varv, in0=ssq, in1=varv, op=OP.subtract)
                cm1 = wt("cm1")
                nc.vector.tensor_scalar(out=v3(cm1), in0=cnt, scalar1=-1.0,
                                        scalar2=1.0, op0=OP.add, op1=OP.max)
                rcm1 = wt("rcm1")
                nc.vector.reciprocal(out=rcm1[:], in_=cm1[:])
                nc.vector.tensor_tensor(out=varv, in0=varv,
                                        in1=rcm1[:].to_broadcast([128, 16, 4]),
                                        op=OP.mult)
                vm = wt("vm")
                nc.vector.tensor_reduce(out=v3(vm), in_=varv, axis=AX.X, op=OP.add)
                nc.vector.tensor_scalar(out=vm[:], in0=vm[:], scalar1=0.25,
                                        scalar2=None, op0=OP.mult)
                phic = wt("phic")
                nc.vector.tensor_scalar(out=phic[:], in0=vm[:], scalar1=2.0,
                                        scalar2=1.0, op0=OP.mult, op1=OP.min)
                nc.vector.tensor_scalar(out=phic[:], in0=phic[:], scalar1=-1.0,
                                        scalar2=1.0, op0=OP.mult, op1=OP.add)
                coh = wt("coh")
                nc.vector.tensor_scalar(out=coh[:], in0=vm[:], scalar1=-1.0,
                                        scalar2=1.0, op0=OP.mult, op1=OP.add)
                perr = wt("perr")
                nc.vector.tensor_tensor(out=v3(perr), in0=ssur, in1=v3(rc),
                                        op=OP.mult)
                integ = wt("integ")
                nc.vector.tensor_scalar(out=integ[:], in0=perr[:], scalar1=-1.0,
                                        scalar2=1.0, op0=OP.mult, op1=OP.add)
                nc.vector.tensor_tensor(out=integ[:], in0=integ[:], in1=phic[:],
                                        op=OP.mult)

                # cluster MLP
                cft = bp.tile([128, 16 * 7], F32, tag="cft")
                cfv = cft[:].rearrange("p (b q) -> p b q", q=7)
                nc.vector.tensor_copy(out=cfv[:, :, 0:4], in_=aggv)
                nc.vector.tensor_copy(out=cfv[:, :, 4:5],
                                      in_=phic[:].to_broadcast([128, 16, 1]))
                nc.vector.tensor_copy(out=cfv[:, :, 5:6],
                                      in_=coh[:].to_broadcast([128, 16, 1]))
                szf = wt("szf")
                nc.vector.tensor_scalar(out=v3(szf), in0=cnt, scalar1=0.05,
                                        scalar2=1.0, op0=OP.mult, op1=OP.min)
                nc.vector.tensor_copy(out=cfv[:, :, 6:7],
                                      in_=szf[:].to_broadcast([128, 16, 1]))
                cftt = bp.tile([7, 2048], F32, tag="cftt")
                for b in range(16):
                    ptc = ppt.tile([128, 128], F32, tag="pt")
                    nc.tensor.transpose(out=ptc[0:7, :],
                                        in_=cft[:, 7 * b:7 * (b + 1)], identity=ids[:])
                    nc.vector.tensor_copy(out=cftt[:, 128 * b:128 * (b + 1)],
                                          in_=ptc[0:7, :])
                hcs = bp.tile([32, 2048], F32, tag="hcs")
                for i in range(4):
                    hcp = ppm.tile([32, 512], F32, tag="mmp")
                    nc.tensor.matmul(out=hcp[:], lhsT=v1s[:],
                                     rhs=cftt[:, 512 * i:512 * (i + 1)],
                                     start=True, stop=True)
                    nc.scalar.activation(out=hcs[:, 512 * i:512 * (i + 1)],
                                         in_=hcp[:], func=AF.Relu, bias=c1s[:])
                sgc = bp.tile([1, 2048], F32, tag="sgc")
                for i in range(4):
                    bcp = ppm.tile([32, 512], F32, tag="mmp")
                    nc.tensor.matmul(out=bcp[0:1, :], lhsT=v2s[:],
                                     rhs=hcs[:, 512 * i:512 * (i + 1)],
                                     start=True, stop=True)
                    nc.scalar.activation(out=sgc[:, 512 * i:512 * (i + 1)],
                                         in_=bcp[0:1, :], func=AF.Sigmoid, bias=c2s[:])
                basec = wt("basec")
                for b in range(16):
                    ptb = ppt.tile([128, 128], F32, tag="pt")
                    nc.tensor.transpose(out=ptb[:, 0:1],
                                        in_=sgc[:, 128 * b:128 * (b + 1)],
                                        identity=ids[0:1, 0:1])
                    nc.vector.tensor_copy(out=basec[:, b:b + 1], in_=ptb[:, 0:1])
                impc = wt("impc")
                nc.vector.tensor_tensor(out=impc[:], in0=basec[:], in1=phic[:],
                                        op=OP.mult)
                nc.vector.tensor_scalar(out=impc[:], in0=impc[:], scalar1=0.01,
                                        scalar2=1.0, op0=OP.max, op1=OP.min)
                valid = wt("valid")
                nc.vector.tensor_scalar(out=v3(valid), in0=cnt, scalar1=0.0,
                                        scalar2=None, op0=OP.is_gt)
                eimp = wt("eimp")
                nc.scalar.activation(out=eimp[:], in_=impc[:], func=AF.Exp)
                nc.vector.tensor_tensor(out=eimp[:], in0=eimp[:], in1=valid[:],
                                        op=OP.mult)
                amx = wt("amx")
                nc.vector.tensor_reduce(out=v3(amx), in_=aggv, axis=AX.X, op=OP.max)
                bsel, bselv = w4("bsel")
                nc.vector.tensor_tensor(out=bselv, in0=aggv,
                                        in1=amx[:].to_broadcast([128, 16, 4]),
                                        op=OP.is_equal)
                taken = wt("taken")
                nc.vector.memset(taken[:], 0.0)
                notk = wt("notk")
                for a in range(4):
                    nc.vector.tensor_scalar(out=notk[:], in0=taken[:], scalar1=-1.0,
                                            scalar2=1.0, op0=OP.mult, op1=OP.add)
                    nc.vector.tensor_tensor(out=bselv[:, :, a:a + 1],
                                            in0=bselv[:, :, a:a + 1], in1=v3(notk),
                                            op=OP.mult)
                    if a < 3:
                        nc.vector.tensor_tensor(out=v3(taken), in0=v3(taken),
                                                in1=bselv[:, :, a:a + 1], op=OP.max)
                # reductions -> R [128, 12]
                r = bp.tile([128, 12], F32, tag="r")
                ga, gav = w4("ga")
                nc.vector.tensor_tensor(out=gav, in0=aggv,
                                        in1=eimp[:].to_broadcast([128, 16, 4]),
                                        op=OP.mult)
                pv = wt("pv")
                nc.vector.tensor_tensor(out=pv[:], in0=phic[:], in1=valid[:],
                                        op=OP.mult)
                cvv = wt("cvv")
                nc.vector.tensor_tensor(out=cvv[:], in0=coh[:], in1=valid[:],
                                        op=OP.mult)
                bv, bvv = w4("bv")
                nc.vector.tensor_tensor(out=bvv, in0=bselv,
                                        in1=valid[:].to_broadcast([128, 16, 4]),
                                        op=OP.mult)
                nc.vector.tensor_reduce(out=r[:, 0:1], in_=eimp[:], axis=AX.X,
                                        op=OP.add)
                gat = ga[:].rearrange("p (b a) -> p a b", a=4)
                nc.vector.tensor_reduce(
                    out=r[:, 1:5].rearrange("p (a o) -> p a o", o=1),
                    in_=gat, axis=AX.X, op=OP.add)
                nc.vector.tensor_reduce(out=r[:, 5:6], in_=pv[:], axis=AX.X, op=OP.add)
                nc.vector.tensor_reduce(out=r[:, 6:7], in_=cvv[:], axis=AX.X,
                                        op=OP.add)
                nc.vector.tensor_reduce(out=r[:, 7:8], in_=valid[:], axis=AX.X,
                                        op=OP.add)
                bvt = bv[:].rearrange("p (b a) -> p a b", a=4)
                nc.vector.tensor_reduce(
                    out=r[:, 8:12].rearrange("p (a o) -> p a o", o=1),
                    in_=bvt, axis=AX.X, op=OP.add)
                orgp = ppm.tile([32, 512], F32, tag="mmp")
                nc.tensor.matmul(out=orgp[0:1, 0:12], lhsT=ones[:], rhs=r[:],
                                 start=True, stop=True)
                orgs = bp.tile([1, 12], F32, tag="orgs")
                nc.vector.tensor_copy(out=orgs[:], in_=orgp[0:1, 0:12])
                nc.sync.dma_start(out=out_org[:], in_=orgs[:])

                # cluster_out [2048, 8]
                oc = bp.tile([128, 128], F32, tag="oc")
                ocv = oc[:].rearrange("p (b q) -> p b q", q=8)
                nc.vector.tensor_copy(out=ocv[:, :, 0:4], in_=aggv)
                nc.vector.tensor_copy(out=ocv[:, :, 4:5],
                                      in_=phic[:].to_broadcast([128, 16, 1]))
                nc.vector.tensor_copy(out=ocv[:, :, 5:6],
                                      in_=coh[:].to_broadcast([128, 16, 1]))
                nc.vector.tensor_copy(out=ocv[:, :, 6:7],
                                      in_=perr[:].to_broadcast([128, 16, 1]))
                nc.vector.tensor_copy(out=ocv[:, :, 7:8],
                                      in_=integ[:].to_broadcast([128, 16, 1]))
                nc.sync.dma_start(
                    out=out_cluster[:].rearrange("(b p) q -> p b q", p=128), in_=ocv)
    return nc


_NC_CACHE = None


def _get_program():
    global _NC_CACHE
    if _NC_CACHE is None:
        _NC_CACHE = build_program()
    return _NC_CACHE


def _host_prep_core(c, state, arch, energy, phi_local, surprise, seg_ids):
    B0 = int(np.searchsorted(seg_ids, 2048 * c))
    B1 = int(np.searchsorted(seg_ids, 2048 * (c + 1)))
    Nc = B1 - B0
    lseg = (seg_ids[B0:B1] - 2048 * c).astype(np.int64)
    idx = np.full(NPAD, -1, np.int64)
    rel = np.full(NPAD, PADSEG, np.float32)
    cur = 0
    for S in range(NTILES):
        blk = S // TPB
        f = _window_start(S)
        wlo = 512 * blk + f
        whi = wlo + W
        take = min(128, int(np.searchsorted(lseg, whi)) - cur)
        if take > 0:
            assert lseg[cur] >= wlo, f"core {c} tile {S}: behind-lag"
            sl = np.arange(cur, cur + take)
            idx[S * 128:S * 128 + take] = sl
            rel[S * 128:S * 128 + take] = (lseg[sl] - wlo).astype(np.float32)
            cur += take
    assert cur == Nc, f"core {c}: {Nc - cur} cells not scheduled"
    m = idx >= 0

    def g(x):
        out = np.zeros((NPAD,) + x.shape[1:], np.float32)
        out[m] = x[B0:B1][idx[m]]
        return out

    return g(state), g(arch), g(energy), g(phi_local), g(surprise), rel


def _swz1(x):
    return x.reshape(NCHUNK, 2, 32, 128).transpose(0, 3, 2, 1).reshape(NCHUNK, 128, 64)


def kernel(state, arch, energy, phi_local, surprise, seg_ids, n_clusters,
           W1, b1, W2, b2, V1, c1, V2, c2):
    state = np.asarray(state, np.float32)
    arch = np.asarray(arch, np.float32)
    energy = np.asarray(energy, np.float32)
    phi_local = np.asarray(phi_local, np.float32)
    surprise = np.asarray(surprise, np.float32)
    seg_ids = np.asarray(seg_ids)
    W1 = np.asarray(W1, np.float32); b1 = np.asarray(b1, np.float32)
    W2 = np.asarray(W2, np.float32); b2 = np.asarray(b2, np.float32)
    V1 = np.asarray(V1, np.float32); c1 = np.asarray(c1, np.float32)
    V2 = np.asarray(V2, np.float32); c2 = np.asarray(c2, np.float32)

    w1d = np.zeros((72, 128), np.float32)
    w1d[0:36, 0:64] = W1
    w1d[36:72, 64:128] = W1
    w2d = np.zeros((128, 2), np.float32)
    w2d[0:64, 0] = W2[:, 0]
    w2d[64:128, 1] = W2[:, 0]
    consts = dict(
        w1d=w1d.astype(ml_dtypes.bfloat16),
        b1d=np.concatenate([b1, b1]).reshape(128, 1).astype(np.float32),
        w2d=w2d.astype(ml_dtypes.bfloat16),
        b2d=np.full((128, 1), b2[0], np.float32),
        iotat=np.ascontiguousarray(
            np.broadcast_to(np.tile(np.arange(W, dtype=np.float32), 128),
                            (128, W * 128))),
        ident=np.eye(128, dtype=np.float32),
        v1=V1, c1b=c1.reshape(32, 1), v2=V2, c2b=c2.reshape(1, 1),
    )
    in_maps = []
    for c in range(NCORES):
        st, ar, en, ph, su, rel = _host_prep_core(
            c, state, arch, energy, phi_local, surprise, seg_ids)
        f36 = np.concatenate([st.T, ar.T], 0)              # [36, NPAD]
        featsT = f36.reshape(36, NCHUNK, 2, 4096).transpose(2, 0, 1, 3).reshape(
            72, NPAD // 2).astype(ml_dtypes.bfloat16)
        acm = ar.reshape(NCHUNK, 2, 32, 128, 4).transpose(0, 3, 2, 1, 4).reshape(
            NCHUNK, 128, 256)
        cvv = np.concatenate([_swz1(en), _swz1(ph), _swz1(su), _swz1(rel)], axis=2)
        in_maps.append(dict(featsT=np.ascontiguousarray(featsT),
                            archcm=np.ascontiguousarray(acm),
                            cellvec=np.ascontiguousarray(cvv), **consts))
    nc = _get_program()
    res = run_bass_kernel_spmd(nc, in_maps, list(range(NCORES)))
    outs = res.results
    couts = [np.asarray(outs[c]["out_cluster"]) for c in range(NCORES)]
    orgs = [np.asarray(outs[c]["out_org"]).reshape(12) for c in range(NCORES)]
    cluster_full = np.concatenate(couts, 0).astype(np.float32)
    p = np.sum(np.stack(orgs, 0), 0, dtype=np.float64)
    Z, G, sphi, scoh, nval, pres = p[0], p[1:5], p[5], p[6], p[7], p[8:12]
    ga = (G / Z).astype(np.float32)
    e = np.exp(ga - ga.max())
    global_arch = (e / e.sum()).astype(np.float32)
    n_valid = max(nval, 1.0)
    avg_phi = sphi / n_valid
    unique = float((pres > 0).sum())
    phi_global = min(1.0, avg_phi * (0.5 + 0.5 * unique / 4.0))
    vert = scoh / n_valid
    self_model = np.array([*global_arch, phi_global, vert], np.float32)
    return np.concatenate([cluster_full.reshape(-1), self_model]).astype(np.float32)



# revision 44
# speedup vs baseline: 72328.6843x; 72328.6843x over previous
"""Trainium2 Bass kernel for nn_BottomUpIntegrator (gnn_message_passing).

Sharding: cells split at cluster boundaries across 8 cores (2048 clusters
each). Per-core segmented reductions via one-hot scatter matmuls accumulating
into 2 rotating PSUM banks (one per active 512-cluster block) with a
core-invariant static window schedule (W=8).

Pipeline (per 8192-cell chunk, software-pipelined across 3 iterations):
  DMA: feats fp8(e3m4) [73,4096] (bias folded as ones-row), host-packed
       bf16 [1|a|sur|a^2|eph] columns + one-hot window rows, one merged DMA.
  PE:  mm1 (h = W1^T f), mm2 (base logits, hs-as-weights trick), scatter
       (vt12 host rows -> PSUM parts 32:42, ew rows -> parts 0:5).
  ACT: relu x4 (tiles q0,2,4,6), tanh (sigmoid = .5+.5 tanh(v/2); Tanh/Exp/
       Relu/Copy share one activation table - no table thrash), exp(w).
  DVE: relu x4 (tiles q1,3,5,7), block drains.
  Pool (SBUF-only): sigmoid affine, clip chain, ew*[1,a] scatter rows.
Cluster phase: strided transposes put cluster 16p+b on partition p (contiguous
output DMA), stats chain + cluster MLP with clusters on partitions (V1 bias
via ones-feature, V2 dot via elementwise+reduce; no transposes back).
Organism-level finale (valid mask, argmax/present, softmax weights) runs on
host in f64 from per-cluster outputs + impc (host knows exact counts).
"""
import numpy as np
import ml_dtypes

import json as _json

from concourse import bass, mybir
from concourse import bass2jax as _b2j
from concourse import bass_utils as _bu
from concourse.tile import TileContext
from concourse.bass_utils import run_bass_kernel_spmd

_orig_compile = _bu.compile_bir_kernel


def _split_waits_compile(bir_json, tmpdir, neff_name="file.neff"):
    """Walrus lowers at most ONE semaphore wait per TPB instruction struct.
    Tile emits several. Hoist extras onto injected same-engine EventSemaphore
    wait instructions immediately before the owner (semantically identical:
    engines execute in program order)."""
    d = _json.loads(bir_json)
    cnt = 0
    for fn in d["functions"]:
        for blk in fn["blocks"]:
            newlist = []
            for ins in blk["instructions"]:
                si = ins.get("sync_info")
                waits = si.get("on_wait", []) if si else []
                if si and len(waits) > 1 and ins.get("opcode") not in (
                        "EventSemaphore",):
                    for w_i, w in enumerate(waits[:-1]):
                        cnt += 1
                        newlist.append({
                            "debug": ins.get("debug", 0),
                            "engine": ins["engine"],
                            "ins": [], "outs": [],
                            "name": f"{ins['name']}-wsplit{w_i}",
                            "opcode": "EventSemaphore",
                            "sync_info": {"on_update": [], "on_wait": [w]},
                        })
                    si["on_wait"] = [waits[-1]]
                newlist.append(ins)
            blk["instructions"] = newlist
    print(f"[wait-split] hoisted {cnt} extra waits")
    return _orig_compile(_json.dumps(d).encode(), tmpdir, neff_name=neff_name)


_bu.compile_bir_kernel = _split_waits_compile
_b2j.compile_bir_kernel = _split_waits_compile

F32 = mybir.dt.float32
BF16 = mybir.dt.bfloat16
FP8 = mybir.dt.float8e3
AF = mybir.ActivationFunctionType
OP = mybir.AluOpType
AX = mybir.AxisListType

NCORES = 8
KLOC = 2048            # clusters per core
NPAD = 262144          # padded cells per core
CHUNK = 8192           # cells per chunk
NCHUNK = NPAD // CHUNK # 32
W = 8                  # onehot window width (clusters)
MARGIN = 2             # window start = clip(s - MARGIN, 0, 512 - W)
NTILES = NPAD // 128   # 2048 scatter tiles per core
TPB = NTILES // 4      # tiles per 512-cluster block
PADSEG = 1.0e9

# chunk barrier period (strict all-engine barrier every N chunks; 0 = never)
BARRIER_EVERY = 4
# relu split of the 4096 h columns per chunk, as slices of the four
# [128,1024] PSUM tiles: (tile, lo, hi, engine, hs_tensor, hs_off)
# A=ACT (bf16 out), P=Pool (f32 out), D=DVE (bf16 out)
# hs is bf16 throughout; relu per 512-col mm1 tile, ACT on even tiles and
# DVE on odd ones (Pool cannot touch PSUM, so only ACT/DVE drain mm1 banks)
HSB_COLS = 4096

# vt16 row layout (stationary rows of the scatter matmul):
#  0     = 1 (count / pad mask)          [host]
#  1:5   = a (4 archetypes)              [host]
#  5     = surprise                      [host]
#  6:10  = a^2                           [host]
#  10    = ew                            [ACT exp -> DVE mult]
#  11:15 = ew * a                        [device]
#  15    = junk (never written/read)
# vt12 (host) col layout: [1, a4, sur, a2_4, eph, pad] (cols 0:10 = vt16 rows 0:10)


def _window_start(S):
    s = S % TPB
    return int(np.clip(s - MARGIN, 0, 512 - W))


def build_program():
    nc = bass.Bass(trn_type="TRN2", use_seq_codegen=True)
    featsT = nc.dram_tensor("featsT", [73, NPAD // 2], FP8, kind="ExternalInput")
    vohd = nc.dram_tensor("vohd", [NCHUNK, 128, 768 + 64 * W], BF16,
                          kind="ExternalInput")
    w1d = nc.dram_tensor("w1d", [73, 128], FP8, kind="ExternalInput")
    cbfd = nc.dram_tensor("cbfd", [128, 674], BF16, kind="ExternalInput")
    cf32d = nc.dram_tensor("cf32d", [128, 130], F32, kind="ExternalInput")
    out_all = nc.dram_tensor("out_all", [128, 144], F32, kind="ExternalOutput")

    with TileContext(nc) as tc:
        with (
            tc.tile_pool(name="const", bufs=1) as cp,
            tc.tile_pool(name="feats", bufs=2) as fp,
            tc.tile_pool(name="hsp", bufs=2) as hp,
            tc.tile_pool(name="small", bufs=2) as sp,
            tc.tile_pool(name="scatv", bufs=2) as vp,
            tc.tile_pool(name="ph_b", bufs=1) as bp,
        ):
            # ---- constants ----------------------------------------------
            w1s = cp.tile([73, 128], FP8, tag="w1s")
            nc.sync.dma_start(out=w1s[:], in_=w1d[:])
            cbf = cp.tile([128, 674], BF16, tag="cbf")
            nc.sync.dma_start(out=cbf[:], in_=cbfd[:])
            cf32 = cp.tile([128, 130], F32, tag="cf32")
            nc.sync.dma_start(out=cf32[:], in_=cf32d[:])
            w2bs = cbf[:, 0:2]
            v2rep = cbf[:, 2:514]
            v1s8 = cbf[0:8, 514:546]
            idb_c = cbf[:, 546:674]
            b2hs = cf32[:, 0:1]
            c2h = cf32[:, 1:2]
            ids = cf32[:, 2:130]
            zbf = cp.tile([128, 512], BF16, tag="zbf")
            nc.vector.memset(zbf[:], 0.0)

            # Pre-touch DMA-loaded constants on their consuming engines so no
            # later compute instruction needs a second (DMA) semaphore wait.
            scra = cp.tile([128, 2], F32, tag="scra")
            nc.scalar.activation(out=scra[:, 0:1], in_=b2hs, func=AF.Copy)
            nc.scalar.activation(out=scra[:, 1:2], in_=c2h, func=AF.Copy)

            # persistent scatter accumulators: 2 rotating PSUM banks [48, 512]
            # dev rows (ew, ewa4) at partitions 0:5, host rows (1, a4, sur,
            # a2_4) at partitions 32:42; zero-initialized by all-zero matmuls.
            pps_cm = tc.tile_pool(name="scatps", bufs=1, space="PSUM")
            pps = pps_cm.__enter__()
            scat = [pps.tile([48, 512], F32, tag=f"scat{b}", name=f"scat{b}")
                    for b in range(2)]
            # PE touch of PE-consumed consts (rides on Ldweights; overwritten
            # by the zeroing matmul below).
            nc.tensor.matmul(out=scat[0][0:1, 0:1], lhsT=ids[0:1, 0:1],
                             rhs=ids[0:1, 0:1], start=True, stop=True,
                             skip_group_check=True)
            nc.tensor.matmul(out=scat[0][0:1, 0:2], lhsT=w1s[0:1, 0:1],
                             rhs=w1s[0:1, 0:2], start=True, stop=True,
                             skip_group_check=True)
            nc.tensor.matmul(out=scat[0][0:2, 0:2], lhsT=w2bs[0:1, :],
                             rhs=w2bs[0:1, :], start=True, stop=True,
                             skip_group_check=True)
            for b in range(2):
                nc.tensor.matmul(out=scat[b][:], lhsT=zbf[:, 0:48], rhs=zbf[:],
                                 start=True, stop=False, skip_group_check=True)
            sc = bp.tile([48, 2048], F32, tag="sc")

            # ---- phase A ------------------------------------------------
            with (
                tc.tile_pool(name="mm1ps", bufs=5, space="PSUM") as pp1,
                tc.tile_pool(name="mm2ps", bufs=1, space="PSUM") as pp2,
            ):
                # software pipeline: at iteration it --
                #   DMA(it) | mm1+relu(it-1) | mm2+chain(it-2) | scatter(it-3)
                T = {}

                def dma_stage(k):
                    ft = fp.tile([73, 4096], FP8, tag="ft")
                    nc.sync.dma_start(out=ft[:],
                                      in_=featsT[:, k * 4096:(k + 1) * 4096])
                    voh = vp.tile([128, 768 + 64 * W], BF16, tag="voh")
                    nc.sync.dma_start(out=voh[:], in_=vohd[k])
                    T[k] = dict(ft=ft, vt12=voh[:, 0:768], oh=voh[:, 768:])

                def mm1_stage(k, half):
                    # half 0: 512-col tiles q0..q3; half 1: q4..q7; relu per
                    # tile alternating ACT/DVE (Pool cannot touch PSUM)
                    d = T[k]
                    if half == 0:
                        d["hsb"] = hp.tile([128, HSB_COLS], BF16, tag="hsb", name=f"hsb{k}")
                    for q in range(4 * half, 4 * half + 4):
                        hp1 = pp1.tile([128, 512], F32, tag="hp1")
                        nc.tensor.matmul(
                            out=hp1[:], lhsT=w1s[:],
                            rhs=d["ft"][:, 512 * q:512 * (q + 1)],
                            start=True, stop=True)
                        dst_ap = d["hsb"][:, 512 * q:512 * (q + 1)]
                        if q % 2 == 0:
                            nc.scalar.activation(out=dst_ap, in_=hp1[:],
                                                 func=AF.Relu)
                        else:
                            nc.vector.tensor_scalar(out=dst_ap, in0=hp1[:],
                                                    scalar1=0.0, scalar2=None,
                                                    op0=OP.max)

                def mm2_stage(k):
                    d = T[k]
                    hsb = d["hsb"]
                    bb = pp2.tile([128, 64], F32, tag="bb")
                    d["bb"] = bb
                    for tt in range(32):
                        nc.tensor.matmul(
                            out=bb[:, 2 * tt:2 * tt + 2],
                            lhsT=hsb[:, 128 * tt:128 * (tt + 1)],
                            rhs=w2bs[:],
                            start=(tt == 0), stop=(tt == 31),
                            skip_group_check=True)

                def chain_a(k):
                    # sig(v) = .5 + .5*tanh(.5*v); x = clip(sig*eph, .01, 1)
                    # w = x*eph; ew = exp(w); vt16 rows 0:10 host, 10:15 dev
                    d = T[k]
                    vt12v = d["vt12"].rearrange("p (s v) -> p s v", v=12)
                    th = sp.tile([128, 64], F32, tag="th")
                    nc.scalar.activation(out=th[:], in_=d["bb"][:], func=AF.Tanh,
                                         bias=b2hs, scale=0.5)
                    sg = sp.tile([128, 64], F32, tag="sg")
                    nc.gpsimd.tensor_scalar(out=sg[:], in0=th[:], scalar1=0.5,
                                            scalar2=0.5, op0=OP.mult, op1=OP.add)
                    xw = sp.tile([128, 128], F32, tag="xw")
                    xv = xw[:, 0:64]
                    wv = xw[:, 64:128]
                    nc.gpsimd.tensor_tensor(
                        out=xv.rearrange("p (s o) -> p s o", o=1),
                        in0=sg[:].rearrange("p (s o) -> p s o", o=1),
                        in1=vt12v[:, :, 10:11], op=OP.mult)
                    nc.gpsimd.tensor_scalar(out=xv, in0=xv, scalar1=0.01,
                                            scalar2=1.0, op0=OP.max, op1=OP.min)
                    nc.gpsimd.tensor_tensor(
                        out=wv.rearrange("p (s o) -> p s o", o=1),
                        in0=xv.rearrange("p (s o) -> p s o", o=1),
                        in1=vt12v[:, :, 10:11], op=OP.mult)
                    d["wv"] = wv

                def chain_b(k):
                    d = T[k]
                    vt12v = d["vt12"].rearrange("p (s v) -> p s v", v=12)
                    ewt = sp.tile([128, 64], BF16, tag="ewt")
                    nc.scalar.activation(out=ewt[:], in_=d["wv"], func=AF.Exp)
                    vt5 = vp.tile([128, 5 * 64], BF16, tag="vt5")
                    v5 = vt5[:].rearrange("p (s v) -> p s v", v=5)
                    nc.gpsimd.tensor_tensor(out=v5,
                                            in0=vt12v[:, :, 0:5],
                                            in1=ewt[:].to_broadcast([128, 64, 5]),
                                            op=OP.mult)
                    d["vt5"] = vt5

                def scatter_stage(k):
                    # col j -> sorted tile S = 64k + 32*(j%2) + j//2
                    d = T[k]
                    sbank = scat[(k // 8) % 2]
                    vt12 = d["vt12"]
                    vt5 = d["vt5"]
                    for j in range(64):
                        S = 64 * k + 32 * (j % 2) + (j // 2)
                        f = _window_start(S)
                        last = (k % 8 == 7 and j >= 62)
                        nc.tensor.matmul(out=sbank[0:5, f:f + W],
                                         lhsT=vt5[:, 5 * j:5 * j + 5],
                                         rhs=d["oh"][:, W * j:W * j + W],
                                         start=False, stop=last,
                                         skip_group_check=True)
                        nc.tensor.matmul(out=sbank[32:42, f:f + W],
                                         lhsT=vt12[:, 12 * j:12 * j + 10],
                                         rhs=d["oh"][:, W * j:W * j + W],
                                         start=False, stop=last,
                                         skip_group_check=True)
                    if k % 8 == 7:
                        blk = k // 8
                        nc.vector.tensor_copy(
                            out=sc[:, 512 * blk:512 * (blk + 1)], in_=sbank[:])
                        if blk < 2:
                            nc.tensor.matmul(out=sbank[:], lhsT=zbf[:, 0:48],
                                             rhs=zbf[:], start=True, stop=False,
                                             skip_group_check=True)
                    # free stale per-chunk state
                    del T[k]

                for it in range(NCHUNK + 3):
                    if it < NCHUNK:
                        dma_stage(it)
                    if 1 <= it <= NCHUNK:
                        mm1_stage(it - 1, 0)
                    if 2 <= it <= NCHUNK + 1:
                        mm2_stage(it - 2)
                        chain_a(it - 2)
                    if 3 <= it <= NCHUNK + 2:
                        scatter_stage(it - 3)
                    if 1 <= it <= NCHUNK:
                        mm1_stage(it - 1, 1)
                    if 2 <= it <= NCHUNK + 1:
                        chain_b(it - 2)

            # ---- phase B ------------------------------------------------
            # sc rows: 0=sew, 1:5=sewa, 32=cnt, 33:37=sa, 37=ssur, 38:42=ssq
            tc.strict_bb_all_engine_barrier()
            pps_cm.__exit__(None, None, None)

            with (
                tc.tile_pool(name="ptps", bufs=4, space="PSUM") as ppt,
                tc.tile_pool(name="ptcs", bufs=3, space="PSUM") as pptc,
                tc.tile_pool(name="mmbps", bufs=1, space="PSUM") as ppm,
            ):
                tt = bp.tile([128, 16 * 48], F32, tag="tt")
                scv = sc[:].rearrange("p (c g) -> p g c", g=16)
                for b in range(16):
                    pt = ppt.tile([128, 48], F32, tag="pt")
                    nc.tensor.transpose(out=pt[:], in_=scv[:, b, :],
                                        identity=ids[0:48, 0:48])
                    if b % 2 == 0:
                        nc.vector.tensor_copy(out=tt[:, 48 * b:48 * (b + 1)],
                                              in_=pt[:])
                    else:
                        nc.scalar.activation(out=tt[:, 48 * b:48 * (b + 1)],
                                             in_=pt[:], func=AF.Copy)
                tv = tt[:].rearrange("p (b q) -> p b q", q=48)
                cnt = tv[:, :, 32:33]    # [128,16,1]
                sa = tv[:, :, 33:37]
                ssur = tv[:, :, 37:38]
                ssq = tv[:, :, 38:42]
                sew = tv[:, :, 0:1]
                sewa = tv[:, :, 1:5]

                def wt(tag):
                    return bp.tile([128, 16], F32, tag=tag, name=tag)

                def v3(t):
                    return t[:].rearrange("p (b a) -> p b a", a=1)

                def w4(tag):
                    t = bp.tile([128, 64], F32, tag=tag, name=tag)
                    return t, t[:].rearrange("p (b a) -> p b a", a=4)

                cntc = wt("cntc")
                nc.vector.tensor_scalar(out=v3(cntc), in0=cnt, scalar1=1.0,
                                        scalar2=None, op0=OP.max)
                rc = wt("rc")
                nc.vector.reciprocal(out=rc[:], in_=cntc[:])
                den = wt("den")
                nc.vector.tensor_scalar(out=v3(den), in0=sew, scalar1=1.0,
                                        scalar2=None, op0=OP.max)
                rden = wt("rden")
                nc.vector.reciprocal(out=rden[:], in_=den[:])
                agr, agrv = w4("agr")
                nc.vector.tensor_tensor(out=agrv, in0=sewa,
                                        in1=rden[:].to_broadcast([128, 16, 4]),
                                        op=OP.mult)
                es, esv = w4("es")
                nc.scalar.activation(out=es[:], in_=agr[:], func=AF.Exp)
                ssum = wt("ssum")
                nc.vector.tensor_reduce(out=v3(ssum), in_=esv, axis=AX.X, op=OP.add)
                rssum = wt("rssum")
                nc.vector.reciprocal(out=rssum[:], in_=ssum[:])
                agg, aggv = w4("agg")
                nc.vector.tensor_tensor(out=aggv, in0=esv,
                                        in1=rssum[:].to_broadcast([128, 16, 4]),
                                        op=OP.mult)
                mean, meanv = w4("mean")
                nc.vector.tensor_tensor(out=meanv, in0=sa,
                                        in1=rc[:].to_broadcast([128, 16, 4]),
                                        op=OP.mult)
                var, varv = w4("var")
                nc.vector.tensor_tensor(out=varv, in0=meanv, in1=meanv, op=OP.mult)
                cntb = wt("cntb")
                nc.vector.tensor_copy(out=v3(cntb), in_=cnt)
                nc.vector.tensor_tensor(out=varv, in0=varv,
                                        in1=cntb[:].to_broadcast([128, 16, 4]),
                                        op=OP.mult)
                nc.vector.tensor_tensor(out=varv, in0=ssq, in1=varv,
                                        op=OP.subtract)
                cm1 = wt("cm1")
                nc.vector.tensor_scalar(out=v3(cm1), in0=cnt, scalar1=-1.0,
                                        scalar2=1.0, op0=OP.add, op1=OP.max)
                rcm1 = wt("rcm1")
                nc.vector.reciprocal(out=rcm1[:], in_=cm1[:])
                nc.vector.tensor_tensor(out=varv, in0=varv,
                                        in1=rcm1[:].to_broadcast([128, 16, 4]),
                                        op=OP.mult)
                vm = wt("vm")
                nc.vector.tensor_reduce(out=v3(vm), in_=varv, axis=AX.X, op=OP.add)
                nc.vector.tensor_scalar(out=vm[:], in0=vm[:], scalar1=0.25,
                                        scalar2=None, op0=OP.mult)
                phic = wt("phic")
                nc.vector.tensor_scalar(out=phic[:], in0=vm[:], scalar1=2.0,
                                        scalar2=1.0, op0=OP.mult, op1=OP.min)
                nc.vector.tensor_scalar(out=phic[:], in0=phic[:], scalar1=-1.0,
                                        scalar2=1.0, op0=OP.mult, op1=OP.add)
                coh = wt("coh")
                nc.vector.tensor_scalar(out=coh[:], in0=vm[:], scalar1=-1.0,
                                        scalar2=1.0, op0=OP.mult, op1=OP.add)
                perr = wt("perr")
                nc.vector.tensor_tensor(out=v3(perr), in0=ssur, in1=v3(rc),
                                        op=OP.mult)
                integ = wt("integ")
                nc.vector.tensor_scalar(out=integ[:], in0=perr[:], scalar1=-1.0,
                                        scalar2=1.0, op0=OP.mult, op1=OP.add)
                nc.vector.tensor_tensor(out=integ[:], in0=integ[:], in1=phic[:],
                                        op=OP.mult)

                # cluster MLP, clusters on partitions throughout:
                # hc = relu(cftt.T @ v1s8) per 128-cluster block, then
                # base = tanh(.5*(hc . v2) + .5*c2) -> sig affine
                cft = bp.tile([128, 16 * 8], BF16, tag="cft")
                cfv = cft[:].rearrange("p (b q) -> p b q", q=8)
                nc.vector.tensor_copy(out=cfv[:, :, 0:4], in_=aggv)
                nc.vector.tensor_copy(out=cfv[:, :, 4:5],
                                      in_=phic[:].to_broadcast([128, 16, 1]))
                nc.vector.tensor_copy(out=cfv[:, :, 5:6],
                                      in_=coh[:].to_broadcast([128, 16, 1]))
                szf = wt("szf")
                nc.vector.tensor_scalar(out=v3(szf), in0=cnt, scalar1=0.05,
                                        scalar2=1.0, op0=OP.mult, op1=OP.min)
                nc.vector.tensor_copy(out=cfv[:, :, 6:7],
                                      in_=szf[:].to_broadcast([128, 16, 1]))
                nc.vector.memset(cfv[:, :, 7:8], 1.0)
                cftt = bp.tile([8, 2048], BF16, tag="cftt")
                for b in range(16):
                    ptc = pptc.tile([128, 128], BF16, tag="ptc")
                    nc.tensor.transpose(out=ptc[0:8, :],
                                        in_=cft[:, 8 * b:8 * (b + 1)],
                                        identity=idb_c)
                    if b % 2 == 0:
                        nc.vector.tensor_copy(out=cftt[:, 128 * b:128 * (b + 1)],
                                              in_=ptc[0:8, :])
                    else:
                        nc.scalar.activation(out=cftt[:, 128 * b:128 * (b + 1)],
                                             in_=ptc[0:8, :], func=AF.Copy)
                hcp = ppm.tile([128, 512], F32, tag="hcp")
                for b in range(16):
                    nc.tensor.matmul(out=hcp[:, 32 * b:32 * (b + 1)],
                                     lhsT=cftt[:, 128 * b:128 * (b + 1)],
                                     rhs=v1s8, start=True, stop=True,
                                     skip_group_check=True)
                hcsb = bp.tile([128, 512], BF16, tag="hcsb")
                nc.scalar.activation(out=hcsb[:], in_=hcp[:], func=AF.Relu)
                hv2 = bp.tile([128, 512], F32, tag="hv2")
                nc.vector.tensor_tensor(out=hv2[:], in0=hcsb[:], in1=v2rep,
                                        op=OP.mult)
                bb2 = wt("bb2")
                nc.vector.tensor_reduce(
                    out=v3(bb2),
                    in_=hv2[:].rearrange("p (b h) -> p b h", h=32),
                    axis=AX.X, op=OP.add)
                basec = wt("basec")
                nc.scalar.activation(out=basec[:], in_=bb2[:], func=AF.Tanh,
                                     bias=c2h, scale=0.5)
                nc.vector.tensor_scalar(out=basec[:], in0=basec[:], scalar1=0.5,
                                        scalar2=0.5, op0=OP.mult, op1=OP.add)

                # cluster_out [2048, 8] + impc, one [128, 144] output tile
                oc = bp.tile([128, 144], F32, tag="oc")
                ocv = oc[:, 0:128].rearrange("p (b q) -> p b q", q=8)
                impc = oc[:, 128:144]
                nc.vector.tensor_tensor(out=impc, in0=basec[:], in1=phic[:],
                                        op=OP.mult)
                nc.vector.tensor_scalar(out=impc, in0=impc, scalar1=0.01,
                                        scalar2=1.0, op0=OP.max, op1=OP.min)
                nc.vector.tensor_copy(out=ocv[:, :, 0:4], in_=aggv)
                nc.vector.tensor_copy(out=ocv[:, :, 4:5],
                                      in_=phic[:].to_broadcast([128, 16, 1]))
                nc.vector.tensor_copy(out=ocv[:, :, 5:6],
                                      in_=coh[:].to_broadcast([128, 16, 1]))
                nc.vector.tensor_copy(out=ocv[:, :, 6:7],
                                      in_=perr[:].to_broadcast([128, 16, 1]))
                nc.vector.tensor_copy(out=ocv[:, :, 7:8],
                                      in_=integ[:].to_broadcast([128, 16, 1]))
                nc.sync.dma_start(out=out_all[:], in_=oc[:])
    return nc


_NC_CACHE = None


def _get_program():
    global _NC_CACHE
    if _NC_CACHE is None:
        _NC_CACHE = build_program()
    return _NC_CACHE


def _host_prep_core(c, state, arch, energy, phi_local, surprise, seg_ids):
    B0 = int(np.searchsorted(seg_ids, 2048 * c))
    B1 = int(np.searchsorted(seg_ids, 2048 * (c + 1)))
    Nc = B1 - B0
    lseg = (seg_ids[B0:B1] - 2048 * c).astype(np.int64)
    idx = np.full(NPAD, -1, np.int64)
    rel = np.full(NPAD, PADSEG, np.float32)
    cur = 0
    for S in range(NTILES):
        blk = S // TPB
        f = _window_start(S)
        wlo = 512 * blk + f
        whi = wlo + W
        take = min(128, int(np.searchsorted(lseg, whi)) - cur)
        if take > 0:
            assert lseg[cur] >= wlo, f"core {c} tile {S}: behind-lag"
            sl = np.arange(cur, cur + take)
            idx[S * 128:S * 128 + take] = sl
            rel[S * 128:S * 128 + take] = (lseg[sl] - wlo).astype(np.float32)
            cur += take
    assert cur == Nc, f"core {c}: {Nc - cur} cells not scheduled"
    m = idx >= 0

    def g(x):
        out = np.zeros((NPAD,) + x.shape[1:], np.float32)
        out[m] = x[B0:B1][idx[m]]
        return out

    return g(state), g(arch), g(energy), g(phi_local), g(surprise), rel, m


def _swz(x):
    """[NPAD, Q] cell-major -> [NCHUNK, 128, 64*Q] device layout."""
    Q = x.shape[1]
    return np.ascontiguousarray(
        x.reshape(NCHUNK, 2, 32, 128, Q).transpose(0, 3, 2, 1, 4).reshape(
            NCHUNK, 128, 64 * Q))


def kernel(state, arch, energy, phi_local, surprise, seg_ids, n_clusters,
           W1, b1, W2, b2, V1, c1, V2, c2):
    state = np.asarray(state, np.float32)
    arch = np.asarray(arch, np.float32)
    energy = np.asarray(energy, np.float32)
    phi_local = np.asarray(phi_local, np.float32)
    surprise = np.asarray(surprise, np.float32)
    seg_ids = np.asarray(seg_ids)
    W1 = np.asarray(W1, np.float32); b1 = np.asarray(b1, np.float32)
    W2 = np.asarray(W2, np.float32); b2 = np.asarray(b2, np.float32)
    V1 = np.asarray(V1, np.float32); c1 = np.asarray(c1, np.float32)
    V2 = np.asarray(V2, np.float32); c2 = np.asarray(c2, np.float32)

    w1d = np.zeros((73, 128), np.float32)
    w1d[0:36, 0:64] = W1
    w1d[36:72, 64:128] = W1
    w1d[72, 0:64] = b1
    w1d[72, 64:128] = b1
    w2f = np.zeros((128, 2), np.float32)
    w2f[0:64, 0] = W2[:, 0]
    w2f[64:128, 1] = W2[:, 0]
    cbf = np.zeros((128, 674), np.float32)
    cbf[:, 0:2] = w2f
    cbf[:, 2:514] = np.tile(V2[:, 0], (128, 16))
    cbf[0:8, 514:546] = np.concatenate([V1, c1.reshape(1, 32)], 0)
    cbf[:, 546:674] = np.eye(128, dtype=np.float32)
    cf32 = np.zeros((128, 130), np.float32)
    cf32[:, 0] = 0.5 * b2[0]
    cf32[:, 1] = 0.5 * c2[0]
    cf32[:, 2:130] = np.eye(128, dtype=np.float32)
    consts = dict(
        w1d=w1d.astype(ml_dtypes.float8_e3m4),
        cbfd=cbf.astype(ml_dtypes.bfloat16),
        cf32d=cf32,
    )
    iw = np.arange(W, dtype=np.float32)

    def _prep(c):
        st, ar, en, ph, su, rel, msk = _host_prep_core(
            c, state, arch, energy, phi_local, surprise, seg_ids)
        f36 = np.concatenate([st.T, ar.T], 0)              # [36, NPAD]
        featsT = np.concatenate(
            [f36.reshape(36, NCHUNK, 2, 4096).transpose(2, 0, 1, 3).reshape(
                72, NPAD // 2),
             np.ones((1, NPAD // 2), np.float32)], 0).astype(ml_dtypes.float8_e3m4)
        # vt12: [1(mask), a4, sur, a2_4, eph, pad]
        vt12 = np.zeros((NPAD, 12), np.float32)
        vt12[:, 0] = msk
        vt12[:, 1:5] = ar
        vt12[:, 5] = su
        vt12[:, 6:10] = ar * ar
        vt12[:, 10] = en * ph
        oh = (rel[:, None] == iw[None, :]).astype(np.float32)   # [NPAD, W]
        voh = np.concatenate([_swz(vt12), _swz(oh)], axis=2)
        return dict(featsT=np.ascontiguousarray(featsT),
                    vohd=np.ascontiguousarray(voh).astype(ml_dtypes.bfloat16),
                    **consts)

    from concurrent.futures import ThreadPoolExecutor
    with ThreadPoolExecutor(NCORES) as ex:
        in_maps = list(ex.map(_prep, range(NCORES)))
    nc = _get_program()
    res = run_bass_kernel_spmd(nc, in_maps, list(range(NCORES)))
    global LAST_RESULT
    LAST_RESULT = res
    outs = res.results
    alls = [np.asarray(outs[c]["out_all"]) for c in range(NCORES)]
    couts = [a[:, 0:128].reshape(2048, 8) for a in alls]
    impcs = [a[:, 128:144].reshape(-1) for a in alls]
    cluster_full = np.concatenate(couts, 0).astype(np.float32)
    impc = np.concatenate(impcs, 0).astype(np.float64)

    # organism-level finale on host (exact, f64)
    K = 16384
    counts = np.bincount(seg_ids, minlength=K)
    valid = counts > 0
    n_valid = max(float(valid.sum()), 1.0)
    aggregate = cluster_full[:, 0:4].astype(np.float64)
    phi_c = cluster_full[:, 4].astype(np.float64)
    coh = cluster_full[:, 5].astype(np.float64)
    iv = np.where(valid, impc, -np.inf)
    e = np.exp(iv - iv.max())
    wc = e / e.sum()
    ga = (wc[:, None] * aggregate).sum(0)
    eg = np.exp(ga - ga.max())
    global_arch = (eg / eg.sum()).astype(np.float32)
    avg_phi = (phi_c * valid).sum() / n_valid
    spec = np.argmax(aggregate, axis=1)
    present = np.zeros(4, bool)
    for a in range(4):
        present[a] = np.any(valid & (spec == a))
    unique = float(present.sum())
    phi_global = min(1.0, avg_phi * (0.5 + 0.5 * unique / 4.0))
    vert = (coh * valid).sum() / n_valid
    self_model = np.array([*global_arch, phi_global, vert], np.float32)
    return np.concatenate([cluster_full.reshape(-1), self_model]).astype(np.float32)
